# revision 6
# baseline (speedup 1.0000x reference)
"""Trainium2 Bass kernel for nn_CogitatDeepSetNorm (segment_reduce).

Math (reference reduces to rank-1 structure):
  rowsum_i = sum_d x[i, d]
  segsum_s = sum_{i: sub_i = s} rowsum_i ;  count_s = |{i: sub_i = s}|
  s_val_s  = relu(Gamma * segsum_s / count_s)            (scalar per segment)
  out[i, :] = relu(Lambda * rowsum_i + 128 * Lambda * s_val_{sub_i})  (bcast to 256)

Distribution: data-parallel over rows across 8 cores; one tiny AllGather of
the per-core [segsum | count] vectors, reduced locally on each core.

Per-core layout: local row r -> (partition p = r % 128, group f = r // 128).
Phase A streams x in 2 MiB tiles, computes rowsums (DVE reduce) and
per-segment (segsum, count) via bf16 one-hot matmuls accumulated in PSUM
[2, 64].  One-hots are built 16 groups at a time with a single bf16
tensor_tensor is_equal against a host-prepared replicated-sub input.
One-hots are exact in bf16; rowsums are rounded to bf16 only on the
segment-mean path (output is ~1e-4 sensitive there); the precision-critical
Lambda*rowsum bias stays f32.
Phase C gathers s_val per row with a bf16 one-hot-transpose matmul that also
broadcasts across the 256 output columns ([128, 256] PSUM f32), then a single
relu-with-per-partition-bias op (ACT and DVE alternating per chunk) produces
each output tile.
"""

import sys

if "/opt/trn_rl_repo" not in sys.path:
    sys.path.insert(0, "/opt/trn_rl_repo")

import numpy as np

N = 131072
D = 256
S = 64          # n_subs
MID = 128       # middle dims
N_CORES = 8
NL = N // N_CORES          # rows per core = 16384
P = 128                    # partitions
F = NL // P                # row-groups per core = 128
CH = 16                    # row-groups per DMA chunk (2 MiB)
NCHUNK = F // CH           # 8

TRACE = False              # test harness sets this for profiling
LAST_RESULT = None         # BassKernelResults of the last run

_build_cache = {}


def _build(gamma: float, lam: float):
    from contextlib import ExitStack

    import concourse.bass as bass
    import concourse.bacc as bacc
    import concourse.tile as tile
    from concourse import mybir

    f32 = mybir.dt.float32
    bf16 = mybir.dt.bfloat16
    Alu = mybir.AluOpType
    Act = mybir.ActivationFunctionType
    X = mybir.AxisListType.X

    nc = bacc.Bacc("TRN2", target_bir_lowering=False, debug=False,
                   num_devices=N_CORES)
    x_d = nc.dram_tensor("x", [NL, D], f32, kind="ExternalInput").ap()
    subf_d = nc.dram_tensor("subf", [NL], f32, kind="ExternalInput").ap()
    subrep_d = nc.dram_tensor("subrep", [P, F, S], bf16,
                              kind="ExternalInput").ap()
    out_d = nc.dram_tensor("out", [NL, D], f32, kind="ExternalOutput").ap()

    with tile.TileContext(nc) as tc, ExitStack() as ctx:
        singles = ctx.enter_context(tc.tile_pool(name="singles", bufs=1))
        xpool = ctx.enter_context(tc.tile_pool(name="xpool", bufs=3))
        ohpool = ctx.enter_context(tc.tile_pool(name="ohpool", bufs=3))
        outpool = ctx.enter_context(tc.tile_pool(name="outpool", bufs=3))
        psa = ctx.enter_context(tc.tile_pool(name="psa", bufs=1, space="PSUM"))
        psc = ctx.enter_context(tc.tile_pool(name="psc", bufs=6, space="PSUM"))
        dram = ctx.enter_context(tc.tile_pool(name="dram", bufs=1, space="DRAM"))

        # ---- constants / small inputs ----
        iota_rep = singles.tile([P, CH, S], bf16)
        nc.gpsimd.iota(iota_rep, pattern=[[0, CH], [1, S]], base=0,
                       channel_multiplier=0,
                       allow_small_or_imprecise_dtypes=True)
        iotac = singles.tile([S, 1], f32)
        nc.gpsimd.iota(iotac, pattern=[[0, 1]], base=0, channel_multiplier=1,
                       allow_small_or_imprecise_dtypes=True)

        ones8 = singles.tile([8, 1], f32)
        nc.vector.memset(ones8, 1.0)
        ones64 = singles.tile([S, D], bf16)
        nc.vector.memset(ones64, 1.0)

        sub_rep = singles.tile([P, F, S], bf16)
        nc.sync.dma_start(out=sub_rep, in_=subrep_d)

        rs_f32 = singles.tile([P, F], f32)          # rowsums (f32, bias path)
        rs2b = singles.tile([P, 2 * F], bf16)        # interleaved [rs, 1] bf16
        nc.vector.memset(rs2b[:, 1:2 * F:2], 1.0)

        psum_seg = psa.tile([2, S], f32)

        # ---- phase A: stream x, rowsums + segment reduce ----
        for n in range(NCHUNK):
            xt = xpool.tile([P, CH, D], f32)
            src = x_d[P * CH * n: P * CH * (n + 1), :].rearrange(
                "(a p) d -> p a d", p=P)
            nc.sync.dma_start(out=xt, in_=src)
            nc.vector.tensor_reduce(
                out=rs_f32[:, CH * n: CH * (n + 1)], in_=xt,
                axis=X, op=Alu.add)
            nc.vector.tensor_copy(
                rs2b[:, 2 * CH * n: 2 * CH * (n + 1): 2],
                rs_f32[:, CH * n: CH * (n + 1)])
            oh = ohpool.tile([P, CH, S], bf16)
            nc.vector.tensor_tensor(out=oh, in0=sub_rep[:, CH * n: CH * (n + 1), :],
                                    in1=iota_rep, op=Alu.is_equal)
            for a in range(CH):
                f = CH * n + a
                nc.tensor.matmul(
                    psum_seg, lhsT=rs2b[:, 2 * f:2 * f + 2], rhs=oh[:, a, :],
                    start=(f == 0), stop=(f == F - 1))

        # ---- sub broadcast (bf16 cast) + one-hot transpose ----
        sub_b = singles.tile([S, NL], bf16)
        sub_bcast_ap = bass.AP(tensor=subf_d.tensor, offset=subf_d.offset,
                               ap=[[0, S], [1, NL]])
        nc.gpsimd.dma_start(out=sub_b, in_=sub_bcast_ap)
        onehotT = singles.tile([S, NL], bf16)
        nc.vector.tensor_scalar(out=onehotT, in0=sub_b, scalar1=iotac,
                                scalar2=None, op0=Alu.is_equal)

        # ---- AllGather of [2, 64] = [segsum | count], reduce locally ----
        seg_sb = singles.tile([2, S], f32)
        nc.vector.tensor_copy(seg_sb, psum_seg)
        cc_in = dram.tile([2, S], f32)
        cc_out = dram.tile([2 * N_CORES, S], f32)
        nc.sync.dma_start(out=cc_in, in_=seg_sb)
        nc.gpsimd.collective_compute(
            "AllGather", Alu.bypass, replica_groups=[list(range(N_CORES))],
            ins=[cc_in.opt()], outs=[cc_out.opt()])
        ag_sb = singles.tile([N_CORES, 2 * S], f32)
        nc.sync.dma_start(
            out=ag_sb, in_=cc_out[:].rearrange("(r two) s -> r (two s)",
                                               two=2))
        psum_red = psa.tile([P, 1], f32)
        nc.tensor.matmul(psum_red, lhsT=ag_sb, rhs=ones8, start=True,
                         stop=True)
        red_sb = singles.tile([P, 1], f32)
        nc.vector.tensor_copy(red_sb, psum_red)

        # ---- s_val = relu(Gamma * segsum / max(count, 1)) * 128 * Lambda ----
        csafe = singles.tile([S, 1], f32)
        nc.vector.tensor_scalar(out=csafe, in0=red_sb[S:2 * S, :],
                                scalar1=1.0, scalar2=None, op0=Alu.max)
        rec = singles.tile([S, 1], f32)
        nc.vector.reciprocal(rec, csafe)
        m_col = singles.tile([S, 1], f32)
        nc.vector.tensor_tensor(out=m_col, in0=red_sb[0:S, :], in1=rec,
                                op=Alu.mult)
        sval = singles.tile([S, 1], f32)
        nc.vector.tensor_scalar(out=sval, in0=m_col, scalar1=float(gamma),
                                scalar2=0.0, op0=Alu.mult, op1=Alu.max)
        sval2 = singles.tile([S, 1], f32)
        nc.vector.tensor_scalar(out=sval2, in0=sval,
                                scalar1=float(MID * lam), scalar2=None,
                                op0=Alu.mult)
        # broadcast along free into bf16 rhs for phase C
        sval_b = singles.tile([S, D], bf16)
        nc.vector.tensor_scalar(out=sval_b, in0=ones64, scalar1=sval2,
                                scalar2=None, op0=Alu.mult)

        # Lambda-scaled rowsums (per-partition bias columns for phase C)
        rs_scaled = singles.tile([P, F], f32)
        nc.vector.tensor_scalar(out=rs_scaled, in0=rs_f32,
                                scalar1=float(lam), scalar2=None,
                                op0=Alu.mult)

        # ---- phase C: gather+broadcast via matmul, relu, store ----
        for n in range(NCHUNK):
            ot = outpool.tile([P, CH, D], f32)
            for a in range(CH):
                f = CH * n + a
                pc = psc.tile([P, D], f32)
                nc.tensor.matmul(
                    pc, lhsT=onehotT[:, P * f:P * (f + 1)],
                    rhs=sval_b, start=True, stop=True)
                if n % 2 == 0:
                    nc.scalar.activation(
                        out=ot[:, a, :], in_=pc, func=Act.Relu,
                        bias=rs_scaled[:, f:f + 1], scale=1.0)
                else:
                    nc.vector.tensor_scalar(
                        out=ot[:, a, :], in0=pc,
                        scalar1=rs_scaled[:, f:f + 1], scalar2=0.0,
                        op0=Alu.add, op1=Alu.max)
            dst = out_d[P * CH * n: P * CH * (n + 1), :].rearrange(
                "(a p) d -> p a d", p=P)
            nc.scalar.dma_start(out=dst, in_=ot)

    nc.compile()
    return nc


def kernel(x, sub, Gamma, Lambda):
    import ml_dtypes
    from concourse import bass_utils

    global LAST_RESULT
    x = np.ascontiguousarray(np.asarray(x, dtype=np.float32))
    sub = np.asarray(sub)
    gamma = float(np.asarray(Gamma).reshape(-1)[0])
    lam = float(np.asarray(Lambda).reshape(-1)[0])

    key = (gamma, lam)
    if key not in _build_cache:
        _build_cache[key] = _build(gamma, lam)
    nc = _build_cache[key]

    sub_f = sub.astype(np.float32)
    in_maps = []
    for c in range(N_CORES):
        sl = slice(c * NL, (c + 1) * NL)
        sub_c = sub_f[sl]
        sub_fp = sub_c.reshape(F, P).T  # [p, f]
        sub_rep = np.ascontiguousarray(
            np.broadcast_to(sub_fp[:, :, None], (P, F, S))
        ).astype(ml_dtypes.bfloat16)
        in_maps.append({
            "x": x[sl],
            "subf": sub_c,
            "subrep": sub_rep,
        })

    res = bass_utils.run_bass_kernel_spmd(
        nc, in_maps, core_ids=list(range(N_CORES)), trace=TRACE)
    LAST_RESULT = res

    out = np.empty((N, D), dtype=np.float32)
    for c in range(N_CORES):
        out[c * NL:(c + 1) * NL] = res.results[c]["out"]
    return out


# revision 7
# speedup vs baseline: 1.2134x; 1.2134x over previous
"""Trainium2 Bass kernel for nn_CogitatDeepSetNorm (segment_reduce).

Math (reference reduces to rank-1 structure):
  rowsum_i = sum_d x[i, d]
  segsum_s = sum_{i: sub_i = s} rowsum_i ;  count_s = |{i: sub_i = s}|
  s_val_s  = relu(Gamma * segsum_s / count_s)            (scalar per segment)
  out[i, :] = relu(Lambda * rowsum_i + 128 * Lambda * s_val_{sub_i})  (bcast to 256)

Distribution: data-parallel over rows across 8 cores; one tiny AllGather of
the per-core [segsum | count] vectors, reduced locally on each core.

Per-core layout: local row r -> (partition p = r // 128, group f = r % 128),
so every x/out DMA is 16 KiB-contiguous per partition (128 fat descriptors).
Phase A streams x in 2 MiB tiles, computes rowsums (DVE reduce) and
per-segment (segsum, count) via bf16 one-hot matmuls accumulated in PSUM
[2, 64].  One-hots are built 16 groups at a time with a single bf16
tensor_tensor is_equal against a host-prepared replicated-sub input.
One-hots are exact in bf16; rowsums are rounded to bf16 only on the
segment-mean path (output is ~1e-4 sensitive there); the precision-critical
Lambda*rowsum bias stays f32.
Phase C gathers s_val per row with a bf16 one-hot-transpose matmul that also
broadcasts across the 256 output columns ([128, 256] PSUM f32), then a single
relu-with-per-partition-bias op (ACT and DVE alternating per chunk) produces
each output tile.
"""

import sys

if "/opt/trn_rl_repo" not in sys.path:
    sys.path.insert(0, "/opt/trn_rl_repo")

import numpy as np

N = 131072
D = 256
S = 64          # n_subs
MID = 128       # middle dims
N_CORES = 8
NL = N // N_CORES          # rows per core = 16384
P = 128                    # partitions
F = NL // P                # row-groups per core = 128
CH = 16                    # row-groups per DMA chunk (2 MiB)
NCHUNK = F // CH           # 8

TRACE = False              # test harness sets this for profiling
LAST_RESULT = None         # BassKernelResults of the last run

_build_cache = {}


def _build(gamma: float, lam: float):
    from contextlib import ExitStack

    import concourse.bass as bass
    import concourse.bacc as bacc
    import concourse.tile as tile
    from concourse import mybir

    f32 = mybir.dt.float32
    bf16 = mybir.dt.bfloat16
    Alu = mybir.AluOpType
    Act = mybir.ActivationFunctionType
    X = mybir.AxisListType.X

    nc = bacc.Bacc("TRN2", target_bir_lowering=False, debug=False,
                   num_devices=N_CORES)
    x_d = nc.dram_tensor("x", [NL, D], f32, kind="ExternalInput").ap()
    subf_d = nc.dram_tensor("subf", [NL], f32, kind="ExternalInput").ap()
    subrep_d = nc.dram_tensor("subrep", [P, F, S], bf16,
                              kind="ExternalInput").ap()
    out_d = nc.dram_tensor("out", [NL, D], f32, kind="ExternalOutput").ap()

    with tile.TileContext(nc) as tc, ExitStack() as ctx:
        singles = ctx.enter_context(tc.tile_pool(name="singles", bufs=1))
        xpool = ctx.enter_context(tc.tile_pool(name="xpool", bufs=3))
        ohpool = ctx.enter_context(tc.tile_pool(name="ohpool", bufs=3))
        outpool = ctx.enter_context(tc.tile_pool(name="outpool", bufs=3))
        psa = ctx.enter_context(tc.tile_pool(name="psa", bufs=1, space="PSUM"))
        psc = ctx.enter_context(tc.tile_pool(name="psc", bufs=6, space="PSUM"))
        dram = ctx.enter_context(tc.tile_pool(name="dram", bufs=1, space="DRAM"))

        # ---- constants / small inputs ----
        iota_rep = singles.tile([P, CH, S], bf16)
        nc.gpsimd.iota(iota_rep, pattern=[[0, CH], [1, S]], base=0,
                       channel_multiplier=0,
                       allow_small_or_imprecise_dtypes=True)
        iotac = singles.tile([S, 1], f32)
        nc.gpsimd.iota(iotac, pattern=[[0, 1]], base=0, channel_multiplier=1,
                       allow_small_or_imprecise_dtypes=True)

        ones8 = singles.tile([8, 1], f32)
        nc.vector.memset(ones8, 1.0)
        ones64 = singles.tile([S, D], bf16)
        nc.vector.memset(ones64, 1.0)

        sub_rep = singles.tile([P, F, S], bf16)
        nc.sync.dma_start(out=sub_rep, in_=subrep_d)

        rs_f32 = singles.tile([P, F], f32)          # rowsums (f32, bias path)
        rs2b = singles.tile([P, 2 * F], bf16)        # interleaved [rs, 1] bf16
        nc.vector.memset(rs2b[:, 1:2 * F:2], 1.0)

        psum_seg = psa.tile([2, S], f32)

        # ---- phase A: stream x, rowsums + segment reduce ----
        for n in range(NCHUNK):
            xt = xpool.tile([P, CH, D], f32)
            src = x_d.rearrange("(p f) d -> p f d", p=P)[:, CH * n: CH * (n + 1), :]
            nc.sync.dma_start(out=xt, in_=src)
            nc.vector.tensor_reduce(
                out=rs_f32[:, CH * n: CH * (n + 1)], in_=xt,
                axis=X, op=Alu.add)
            nc.vector.tensor_copy(
                rs2b[:, 2 * CH * n: 2 * CH * (n + 1): 2],
                rs_f32[:, CH * n: CH * (n + 1)])
            oh = ohpool.tile([P, CH, S], bf16)
            nc.vector.tensor_tensor(out=oh, in0=sub_rep[:, CH * n: CH * (n + 1), :],
                                    in1=iota_rep, op=Alu.is_equal)
            for a in range(CH):
                f = CH * n + a
                nc.tensor.matmul(
                    psum_seg, lhsT=rs2b[:, 2 * f:2 * f + 2], rhs=oh[:, a, :],
                    start=(f == 0), stop=(f == F - 1))

        # ---- sub broadcast (bf16 cast) + one-hot transpose ----
        sub_b = singles.tile([S, NL], bf16)
        sub_bcast_ap = bass.AP(tensor=subf_d.tensor, offset=subf_d.offset,
                               ap=[[0, S], [1, NL]])
        nc.gpsimd.dma_start(out=sub_b, in_=sub_bcast_ap)
        onehotT = singles.tile([S, NL], bf16)
        nc.vector.tensor_scalar(out=onehotT, in0=sub_b, scalar1=iotac,
                                scalar2=None, op0=Alu.is_equal)

        # ---- AllGather of [2, 64] = [segsum | count], reduce locally ----
        seg_sb = singles.tile([2, S], f32)
        nc.vector.tensor_copy(seg_sb, psum_seg)
        cc_in = dram.tile([2, S], f32)
        cc_out = dram.tile([2 * N_CORES, S], f32)
        nc.sync.dma_start(out=cc_in, in_=seg_sb)
        nc.gpsimd.collective_compute(
            "AllGather", Alu.bypass, replica_groups=[list(range(N_CORES))],
            ins=[cc_in.opt()], outs=[cc_out.opt()])
        ag_sb = singles.tile([N_CORES, 2 * S], f32)
        nc.sync.dma_start(
            out=ag_sb, in_=cc_out[:].rearrange("(r two) s -> r (two s)",
                                               two=2))
        psum_red = psa.tile([P, 1], f32)
        nc.tensor.matmul(psum_red, lhsT=ag_sb, rhs=ones8, start=True,
                         stop=True)
        red_sb = singles.tile([P, 1], f32)
        nc.vector.tensor_copy(red_sb, psum_red)

        # ---- s_val = relu(Gamma * segsum / max(count, 1)) * 128 * Lambda ----
        csafe = singles.tile([S, 1], f32)
        nc.vector.tensor_scalar(out=csafe, in0=red_sb[S:2 * S, :],
                                scalar1=1.0, scalar2=None, op0=Alu.max)
        rec = singles.tile([S, 1], f32)
        nc.vector.reciprocal(rec, csafe)
        m_col = singles.tile([S, 1], f32)
        nc.vector.tensor_tensor(out=m_col, in0=red_sb[0:S, :], in1=rec,
                                op=Alu.mult)
        sval = singles.tile([S, 1], f32)
        nc.vector.tensor_scalar(out=sval, in0=m_col, scalar1=float(gamma),
                                scalar2=0.0, op0=Alu.mult, op1=Alu.max)
        sval2 = singles.tile([S, 1], f32)
        nc.vector.tensor_scalar(out=sval2, in0=sval,
                                scalar1=float(MID * lam), scalar2=None,
                                op0=Alu.mult)
        # broadcast along free into bf16 rhs for phase C
        sval_b = singles.tile([S, D], bf16)
        nc.vector.tensor_scalar(out=sval_b, in0=ones64, scalar1=sval2,
                                scalar2=None, op0=Alu.mult)

        # Lambda-scaled rowsums (per-partition bias columns for phase C)
        rs_scaled = singles.tile([P, F], f32)
        nc.vector.tensor_scalar(out=rs_scaled, in0=rs_f32,
                                scalar1=float(lam), scalar2=None,
                                op0=Alu.mult)

        # ---- phase C: gather+broadcast via matmul, relu, store ----
        for n in range(NCHUNK):
            ot = outpool.tile([P, CH, D], f32)
            for a in range(CH):
                f = CH * n + a
                pc = psc.tile([P, D], f32)
                nc.tensor.matmul(
                    pc, lhsT=onehotT[:, P * f:P * (f + 1)],
                    rhs=sval_b, start=True, stop=True)
                if n % 2 == 0:
                    nc.scalar.activation(
                        out=ot[:, a, :], in_=pc, func=Act.Relu,
                        bias=rs_scaled[:, f:f + 1], scale=1.0)
                else:
                    nc.vector.tensor_scalar(
                        out=ot[:, a, :], in0=pc,
                        scalar1=rs_scaled[:, f:f + 1], scalar2=0.0,
                        op0=Alu.add, op1=Alu.max)
            dst = out_d.rearrange("(p f) d -> p f d", p=P)[:, CH * n: CH * (n + 1), :]
            nc.scalar.dma_start(out=dst, in_=ot)

    nc.compile()
    return nc


def kernel(x, sub, Gamma, Lambda):
    import ml_dtypes
    from concourse import bass_utils

    global LAST_RESULT
    x = np.ascontiguousarray(np.asarray(x, dtype=np.float32))
    sub = np.asarray(sub)
    gamma = float(np.asarray(Gamma).reshape(-1)[0])
    lam = float(np.asarray(Lambda).reshape(-1)[0])

    key = (gamma, lam)
    if key not in _build_cache:
        _build_cache[key] = _build(gamma, lam)
    nc = _build_cache[key]

    sub_f = sub.astype(np.float32)
    in_maps = []
    for c in range(N_CORES):
        sl = slice(c * NL, (c + 1) * NL)
        sub_c = sub_f[sl]
        sub_pf = sub_c.reshape(P, F)     # row r = p*F + f
        sub_rep = np.ascontiguousarray(
            np.broadcast_to(sub_pf[:, :, None], (P, F, S))
        ).astype(ml_dtypes.bfloat16)
        in_maps.append({
            "x": x[sl],
            "subf": np.ascontiguousarray(sub_pf.T).reshape(-1),  # f-major
            "subrep": sub_rep,
        })

    res = bass_utils.run_bass_kernel_spmd(
        nc, in_maps, core_ids=list(range(N_CORES)), trace=TRACE)
    LAST_RESULT = res

    out = np.empty((N, D), dtype=np.float32)
    for c in range(N_CORES):
        out[c * NL:(c + 1) * NL] = res.results[c]["out"]
    return out
